# revision 28
# baseline (speedup 1.0000x reference)
"""Trainium2 Bass kernel for AdaptivePrototypeContrastiveLoss.

Strategy
--------
Host (cheap, O(N*D) bookkeeping, all f64):
  * closed-form momentum EMA + LAPACK QR -> new prototypes  [7,256]
  * row-normalize feats, stable-sort rows by label
  * per-core COLUMN PERMUTATION: core k's 1024 rows span 1-2 classes;
    its gram columns are laid out [A: na cols of other classes |
    B1: nb cols = own class 1 | B2: nb cols = own class 2 or the
    non-own spill]; na/nb are trimmed to the actual label stats at
    build time (identical across cores, so the graph stays SPMD).
    Every row's negative-sum is A + sel1*B1 + sel2*B2 with per-row
    0/1 sels and a pad correction.
  * the positive term pos_i = f_i . Y_class(i), ln, threshold and the
    final mean are all computed on host.

Device (8 NeuronCores, SPMD, no collectives; only the O(N^2) work):
  * row-shard: each core owns 8 row-tiles of 128 rows; the last 7 rows'
    column sweep (row-tile 64) is split column-wise across all 8 cores
    and runs FIRST, inside the input-DMA shadow
  * per row-tile: 6 supertiles  B1(nb) B2(nb) A(~na/4)x4; G = rows@ft^T
    via PE (fp8-e4m3 DoubleRow, f32 PSUM, <=512-col chunks, two
    pingponged [128,1536] psum tiles).  Every matmul group (~1.1us)
    hides under every exp (>=1.2us) -> near-gap-free ACT pipeline.
  * ACT computes exp(A*sim + BIAS) PSUM->bf16 scratch with fused
    accum_out: 6 instructions/tile write the 6 block sums straight
    into the output tile — no DVE column reductions, no epilogue
  * the global max subtraction is replaced by the constant M0=12.5
    (max only enters through ~1e-8-scale eps terms, verified offline)
  * inputs ride 4 fat contiguous DMAs ordered by first use; the ACT
    exp table is pre-warmed by a dummy exp during the DMA shadow
  * per-core output: [128, 56] raw accumulator scalars, one DMA
Host: negsum algebra in f64, ln, pos, threshold, mean -> scalar.
"""

import numpy as np

import concourse.bass as bass
import concourse.tile as tile
from concourse import mybir
from concourse.bass_utils import run_bass_kernel_spmd

# ---- problem constants (hardcoded per spec) ----
TEMP = 0.08
EPS = 1e-8
GAMMA = 0.99
BETA = 0.5 * (1.0 - GAMMA)
B, D, C = 8192, 256, 7
N = B + C                      # 8199 rows/cols of the score matrix
NCORES = 8
NT = 8                         # full row-tiles per core (8*8*128 = 8192)
ROWS_PER_CORE = NT * 128       # 1024
NA = 6144                      # block-A width (non-own columns)
NBMAX = 1536                   # hard cap on block-B width (psum tile)
SUPER = 2048                   # ftA piece width (= first A-block half)
NSLOT = 6                      # accum slots per row-tile: A1..A4 B1 B2
WA = 1536                      # A-supertile exp width
T8W = 1536                     # per-core share of row-tile 64's columns
OUTW = 56                      # NT*NSLOT main accums + 3 t64 accums + pad
M0 = 12.5                      # constant stand-in for the global max
A_SCALE = 0.5 / float(np.float32(TEMP))
BIAS = (0.5 + EPS) / float(np.float32(TEMP)) - M0
EXPBIAS = float(np.exp(np.float64(np.float32(BIAS))))  # one pad col's exp

F32 = mybir.dt.float32
BF16 = mybir.dt.bfloat16
FP8 = mybir.dt.float8e4
FP8NP = mybir.dt.np(mybir.dt.float8e4)
ACTF = mybir.ActivationFunctionType


def _split_multi_waits(nc):
    """This container's walrus accepts only ONE sync wait per instruction;
    split extra waits into standalone single-wait EventSemaphore insts."""
    n_new = 0
    for func in nc.m.functions:
        for blk in func.blocks:
            new_insts = []
            for inst in blk.instructions:
                si = getattr(inst, "sync_info", None)
                waits = list(si.on_wait) if si and si.on_wait else []
                if len(waits) > 1:
                    for i, w in enumerate(waits[:-1]):
                        n_new += 1
                        ev = mybir.InstEventSemaphore(
                            name=f"{inst.name}-wsplit{i}",
                            engine=inst.engine,
                            ins=[],
                            outs=[],
                            sync_info=mybir.SyncInfo(on_wait=[w], on_update=[]),
                            bass_nofuse=True,
                        )
                        new_insts.append(ev)
                    si.on_wait = [waits[-1]]
                new_insts.append(inst)
            blk.instructions = new_insts
    return n_new


def _pack_kt(mat):
    """[cols, D] f32 -> [128, 2, cols] fp8 DoubleRow operand layout."""
    cols = mat.shape[0]
    return np.ascontiguousarray(
        mat.T.reshape(2, 128, cols).transpose(1, 0, 2)
    ).astype(FP8NP)


def _host_prep(features, labels, prototypes, momentums):
    features = np.asarray(features, dtype=np.float32)
    labels = np.asarray(labels).astype(np.int64)
    prototypes = np.asarray(prototypes, dtype=np.float32)
    momentums = np.asarray(momentums, dtype=np.float32)

    # ---- prototype update: closed form of the sequential EMA scan ----
    counts_feat = np.bincount(labels, minlength=C)
    rank = np.zeros(B, dtype=np.int64)
    seen = np.zeros(C, dtype=np.int64)
    for i, l in enumerate(labels):
        rank[i] = seen[l]
        seen[l] += 1
    w = BETA * (GAMMA ** (counts_feat[labels] - 1 - rank).astype(np.float64))
    S = np.zeros((C, B))
    S[labels, np.arange(B)] = w
    m_final = S @ features.astype(np.float64)
    wsum = np.bincount(labels, weights=w, minlength=C)
    m_final -= wsum[:, None] * prototypes.astype(np.float64)
    m_final += (GAMMA ** counts_feat.astype(np.float64))[:, None] * momentums.astype(
        np.float64
    )
    target = prototypes.astype(np.float64) + m_final
    q, _ = np.linalg.qr(target.T.astype(np.float32))
    new_protos = q.T.astype(np.float32)

    # ---- normalized, label-sorted gram operands ----
    feats = np.concatenate([features, new_protos], 0)
    labs = np.concatenate([labels, np.arange(C, dtype=np.int64)])
    nrm = np.linalg.norm(feats.astype(np.float64), axis=-1)
    fhat = feats.astype(np.float64) / nrm[:, None]
    perm = np.argsort(labs, kind="stable")
    fs = fhat[perm]
    ls = labs[perm]
    counts_all = np.bincount(ls, minlength=C)          # includes protos
    bounds = np.concatenate([[0], np.cumsum(counts_all)])  # class col ranges

    fs32 = fs.astype(np.float32)
    Y = np.zeros((D, C), dtype=np.float64)
    for c in range(C):
        Y[:, c] = fs[bounds[c]:bounds[c + 1]].sum(0)

    # ---- shared row-tile 64 (last 7 rows), column-split across cores ----
    t8pad = np.zeros((128, D), dtype=np.float32)
    t8pad[:7] = fs32[B:N]
    t8rows = _pack_kt(t8pad)
    chunk_cols = []  # class-pure 512-col chunks (global col indices)
    chunk_cls = []
    for c in range(C):
        cols = np.arange(bounds[c], bounds[c + 1])
        for o in range(0, len(cols), 512):
            chunk_cols.append(cols[o:o + 512])
            chunk_cls.append(c)
    n_cpc = T8W // 512  # chunks per core
    while len(chunk_cols) < NCORES * n_cpc:
        chunk_cols.append(np.zeros(0, dtype=np.int64))
        chunk_cls.append(-1)
    t8meta = []  # (class, n_pad) per chunk for the host-side combine
    t8cols_per_core = []
    for core in range(NCORES):
        colmat = np.zeros((T8W, D), dtype=np.float32)
        for j in range(n_cpc):
            ci = core * n_cpc + j
            cols = chunk_cols[ci]
            colmat[j * 512:j * 512 + len(cols)] = fs32[cols]
            t8meta.append((chunk_cls[ci], 512 - len(cols)))
        t8cols_per_core.append(_pack_kt(colmat))

    nb = int(counts_all.max())          # B-block width, trimmed to the data
    assert nb <= NBMAX
    # A-block width: what the neediest core requires (two-class cores hold
    # all non-own cols in A; one-class cores may spill up to nb into B2)
    need_a = []
    for core in range(NCORES):
        own = sorted(set(ls[core * ROWS_PER_CORE:(core + 1) * ROWS_PER_CORE]
                         .tolist()))
        nonown = N - sum(int(counts_all[c]) for c in own)
        need_a.append(nonown if len(own) == 2 else nonown - nb)
    na = (max(need_a) + 15) // 16 * 16
    assert na <= NA
    wa = (na // 4 + 15) // 16 * 16     # A supertile width (first 3 supers)
    npad = na + 2 * nb
    per_core = []
    layouts = []
    for core in range(NCORES):
        base = core * ROWS_PER_CORE
        row_ls = ls[base:base + ROWS_PER_CORE]
        own = sorted(set(int(x) for x in row_ls))
        assert len(own) <= 2, f"core {core} spans {len(own)} classes"
        c1 = own[0]
        c2 = own[1] if len(own) == 2 else -1

        own_mask = np.isin(ls, own)
        nonown_cols = np.nonzero(~own_mask)[0]
        c1_cols = np.arange(bounds[c1], bounds[c1 + 1])
        if c2 >= 0:
            c2_cols = np.arange(bounds[c2], bounds[c2 + 1])
            a_cols = nonown_cols
        else:
            a_cols = nonown_cols[:na]
            c2_cols = nonown_cols[na:]  # non-own spill
        assert len(a_cols) <= na and len(c1_cols) <= nb and len(c2_cols) <= nb
        pads = (na - len(a_cols), nb - len(c1_cols), nb - len(c2_cols))

        ftpad = np.zeros((npad, D), dtype=np.float32)
        ftpad[0:len(a_cols)] = fs32[a_cols]
        ftpad[na:na + len(c1_cols)] = fs32[c1_cols]
        ftpad[na + nb:na + nb + len(c2_cols)] = fs32[c2_cols]
        ft = _pack_kt(ftpad)

        per_core.append(
            {
                "m0": np.ascontiguousarray(np.concatenate(
                    [ft[:, :, 0:wa],
                     _pack_kt(fs32[base:base + ROWS_PER_CORE])], axis=2)),
                "ftAB": np.ascontiguousarray(ft[:, :, wa:na]),
                "ftBB": np.ascontiguousarray(ft[:, :, na:]),
                "t8": np.ascontiguousarray(
                    np.concatenate([t8cols_per_core[core], t8rows], axis=2)),
            }
        )
        layouts.append((c1, c2, pads))

    host = {
        "ls": ls, "bounds": bounds, "counts_all": counts_all, "fs": fs,
        "Y": Y, "t8meta": t8meta, "layouts": layouts, "nb": nb, "na": na,
        "wa": wa,
    }
    return per_core, host


def _build_graph(na, nb, wa):
    npad = na + 2 * nb
    wa4 = na - 3 * wa
    nc = bass.Bass()
    m0_d = nc.declare_dram_parameter(
        "m0", [128, 2, wa + ROWS_PER_CORE], FP8, isOutput=False
    )
    ftAB_d = nc.declare_dram_parameter(
        "ftAB", [128, 2, na - wa], FP8, isOutput=False
    )
    ftBB_d = nc.declare_dram_parameter(
        "ftBB", [128, 2, npad - na], FP8, isOutput=False
    )
    t8_d = nc.declare_dram_parameter("t8", [128, 2, T8W + 128], FP8, isOutput=False)
    out_d = nc.declare_dram_parameter("out", [128, OUTW], F32, isOutput=True)

    with tile.TileContext(nc) as tc:
        with (
            tc.tile_pool(name="persist", bufs=1) as persist,
            tc.tile_pool(name="ps", bufs=2, space="PSUM") as psA,
            tc.tile_pool(name="scr", bufs=3) as scrp,
        ):
            # --- resident inputs (contiguous per-partition = fat DMAs) ---
            # order: t64 operands first (fills the DMA shadow), then the
            # pieces in first-use order for tile 0 = A1..A4 B1 B2
            t8_sb = persist.tile([128, 2, T8W + 128], FP8, tag="t8")
            nc.sync.dma_start(out=t8_sb[:], in_=t8_d[:])
            t8c_sb = t8_sb[:, :, 0:T8W]
            t8r_sb = t8_sb[:, :, T8W:T8W + 128]
            m0_sb = persist.tile([128, 2, wa + ROWS_PER_CORE], FP8, tag="m0")
            nc.sync.dma_start(out=m0_sb[:], in_=m0_d[:])
            ftA_sb = m0_sb[:, :, 0:wa]
            rows_sb = m0_sb[:, :, wa:wa + ROWS_PER_CORE]
            ftAB_sb = persist.tile([128, 2, na - wa], FP8, tag="ftAB")
            nc.sync.dma_start(out=ftAB_sb[:], in_=ftAB_d[:])
            ftBB_sb = persist.tile([128, 2, npad - na], FP8, tag="ftBB")
            nc.sync.dma_start(out=ftBB_sb[:], in_=ftBB_d[:])

            bias_exp = persist.tile([128, 1], F32, tag="bias_exp")
            nc.vector.memset(bias_exp[:], float(BIAS))
            out_t = persist.tile([128, OUTW], F32, tag="out")

            # warm the ACT exp table inside the DMA shadow
            dummy = persist.tile([128, 1], F32, tag="dummy")
            nc.scalar.activation(dummy[:], bias_exp[:], ACTF.Exp, bias=0.0)

            def rhs(lo, hi):
                if hi <= wa:
                    return ftA_sb[:, :, lo:hi]
                if hi <= na:
                    return ftAB_sb[:, :, lo - wa:hi - wa]
                return ftBB_sb[:, :, lo - na:hi - na]

            # --- shared row-tile 64 first: fills the input-DMA shadow ---
            ps8 = psA.tile([128, WA], F32, tag="ps")
            for j in range(T8W // 512):
                nc.tensor.matmul(
                    ps8[:, j * 512:(j + 1) * 512],
                    lhsT=t8r_sb[:],
                    rhs=t8c_sb[:, :, j * 512:(j + 1) * 512],
                    start=True,
                    stop=True,
                    perf_mode=mybir.MatmulPerfMode.DoubleRow,
                )
            scr8 = scrp.tile([128, WA], BF16, tag="scr")
            nc.scalar.activation(
                scr8[:, 0:T8W], ps8[:, 0:T8W],
                ACTF.Exp, bias=bias_exp[:], scale=float(A_SCALE),
            )
            for j in range(T8W // 512):
                nc.vector.reduce_sum(
                    out_t[:, NT * NSLOT + j:NT * NSLOT + j + 1],
                    scr8[:, j * 512:(j + 1) * 512], mybir.AxisListType.X,
                )

            # --- main loop over row-tiles ---
            # per tile: 6 supertiles  B1(1280) B2(1280) A(1536)x4 — every
            # matmul group (<=3 chunks) hides under every exp (>=1280 wide)
            sup_w = (nb, nb, wa, wa, wa, wa4)
            sup_lo = (na, na + nb, 0, wa, 2 * wa, 3 * wa)
            slot_of = (4, 5, 0, 1, 2, 3)  # accum slot per super (host order)
            for t in range(NT):
                lhsT = rows_sb[:, :, t * 128:(t + 1) * 128]
                order = (2, 3, 4, 5, 0, 1) if t == 0 else range(NSLOT)
                for s in order:
                    ps = psA.tile([128, WA], F32, tag="ps")
                    lo, w = sup_lo[s], sup_w[s]
                    for c in range(0, w, 512):
                        cw = min(512, w - c)
                        nc.tensor.matmul(
                            ps[:, c:c + cw],
                            lhsT=lhsT,
                            rhs=rhs(lo + c, lo + c + cw),
                            start=True,
                            stop=True,
                            perf_mode=mybir.MatmulPerfMode.DoubleRow,
                        )
                    scr = scrp.tile([128, WA], BF16, tag="scr")
                    k = t * NSLOT + slot_of[s]
                    if slot_of[s] in (0, 1):  # A1, A2 keep the ACT accum
                        nc.scalar.activation(
                            scr[:, 0:w], ps[:, 0:w], ACTF.Exp,
                            bias=bias_exp[:], scale=float(A_SCALE),
                            accum_out=out_t[:, k:k + 1],
                        )
                    else:  # B1, B2, A3, A4: reduce on the idle Vector engine
                        nc.scalar.activation(
                            scr[:, 0:w], ps[:, 0:w], ACTF.Exp,
                            bias=bias_exp[:], scale=float(A_SCALE),
                        )
                        nc.vector.reduce_sum(
                            out_t[:, k:k + 1], scr[:, 0:w], mybir.AxisListType.X
                        )

            nc.sync.dma_start(out=out_d[:], in_=out_t[:])
    return nc


def _combine(results, host):
    """Host-side unshard: all O(N) math in f64."""
    ls = host["ls"]
    fs, Y = host["fs"], host["Y"]
    counts_all = host["counts_all"]
    cnt = (counts_all[ls] - 1).astype(np.float64)
    selfsim = (fs.astype(np.float64) ** 2).sum(1)

    # pos_i = (A*(f_i . Y_c - selfsim_i) + BIAS*cnt_i) / (cnt_i + EPS)
    pos_sel = np.einsum("id,di->i", fs, Y[:, ls])
    pos = (A_SCALE * (pos_sel - selfsim) + BIAS * cnt) / (cnt + EPS)

    # negsum for main rows from per-core accumulators
    negsum = np.zeros(N, dtype=np.float64)
    for core in range(NCORES):
        o = np.asarray(results[core]["out"], dtype=np.float64)
        acc = o[:, :NT * NSLOT].reshape(128, NT, NSLOT)
        c1, c2, (pa, pb1, pb2) = host["layouts"][core]
        base = core * ROWS_PER_CORE
        row_ls = ls[base:base + ROWS_PER_CORE].reshape(NT, 128).T  # [128, NT]
        sel1 = (row_ls != c1).astype(np.float64)
        sel2 = (
            np.ones_like(sel1) if c2 < 0 else (row_ls != c2).astype(np.float64)
        )
        ns = (
            acc[:, :, 0] + acc[:, :, 1] + acc[:, :, 2] + acc[:, :, 3]
            + sel1 * acc[:, :, 4] + sel2 * acc[:, :, 5]
            - (pa + sel1 * pb1 + sel2 * pb2) * EXPBIAS
        )
        negsum[base:base + ROWS_PER_CORE] = ns.T.reshape(-1)

    # row-tile 64: rows 8192..8198 — class sums from per-core chunk sums
    n7 = N - B
    n_cpc = T8W // 512
    classsum = np.zeros((n7, C), dtype=np.float64)
    for core in range(NCORES):
        o = np.asarray(results[core]["out"], dtype=np.float64)
        for j in range(n_cpc):
            cls, n_pad = host["t8meta"][core * n_cpc + j]
            if cls < 0:
                continue
            classsum[:, cls] += o[:n7, NT * NSLOT + j] - n_pad * EXPBIAS
    stot = classsum.sum(1)
    rows_ls = ls[B:N]
    negsum[B:N] = stot - classsum[np.arange(n7), rows_ls]

    loss = -pos + np.log(negsum + EPS)
    m = loss > 0
    cnt_sum = m.sum()
    val = loss[m].sum() / max(cnt_sum, 1.0) if cnt_sum > 0 else 0.0
    return np.float32(val)


def _run(features, labels, prototypes, momentums, trace=False, trace_kwargs=None):
    per_core, host = _host_prep(features, labels, prototypes, momentums)
    nc = _build_graph(host["na"], host["nb"], host["wa"])
    _split_multi_waits(nc)
    in_maps = [per_core[i] for i in range(NCORES)]
    kw = {}
    if trace:
        kw = dict(trace=True, trace_cores=list(range(NCORES)))
        if trace_kwargs:
            kw["trace_kwargs"] = trace_kwargs
    res = run_bass_kernel_spmd(nc, in_maps, core_ids=list(range(NCORES)), **kw)
    return _combine(res.results, host), res


def kernel(features, labels, prototypes, momentums):
    val, _ = _run(features, labels, prototypes, momentums)
    return np.array(val, dtype=np.float32)


# revision 29
# speedup vs baseline: 1.1355x; 1.1355x over previous
"""Trainium2 Bass kernel for AdaptivePrototypeContrastiveLoss.

Strategy
--------
Host (cheap, O(N*D) bookkeeping, all f64):
  * closed-form momentum EMA + LAPACK QR -> new prototypes  [7,256]
  * row-normalize feats, stable-sort rows by label
  * per-core COLUMN PERMUTATION: core k's 1024 rows span 1-2 classes;
    its gram columns are laid out [A: na cols of other classes |
    B1: nb cols = own class 1 | B2: nb cols = own class 2 or the
    non-own spill]; na/nb are trimmed to the actual label stats at
    build time (identical across cores, so the graph stays SPMD).
    Every row's negative-sum is A + sel1*B1 + sel2*B2 with per-row
    0/1 sels and a pad correction.
  * the positive term pos_i = f_i . Y_class(i), ln, threshold and the
    final mean are all computed on host.

Device (8 NeuronCores, SPMD, no collectives; only the O(N^2) work):
  * row-shard: each core owns 8 row-tiles of 128 rows; the last 7 rows'
    column sweep (row-tile 64) is split column-wise across all 8 cores
    and runs FIRST, inside the input-DMA shadow
  * per row-tile: 6 supertiles  B1(nb) B2(nb) A(~na/4)x4; G = rows@ft^T
    via PE (fp8-e4m3 DoubleRow, f32 PSUM, <=512-col chunks, two
    pingponged [128,1536] psum tiles).  Every matmul group (~1.1us)
    hides under every exp (>=1.2us) -> near-gap-free ACT pipeline.
  * ACT computes exp(A*sim + BIAS) PSUM->bf16 scratch with fused
    accum_out: 6 instructions/tile write the 6 block sums straight
    into the output tile — no DVE column reductions, no epilogue
  * the global max subtraction is replaced by the constant M0=12.5
    (max only enters through ~1e-8-scale eps terms, verified offline)
  * inputs ride 4 fat contiguous DMAs ordered by first use; the ACT
    exp table is pre-warmed by a dummy exp during the DMA shadow
  * per-core output: [128, 56] raw accumulator scalars, one DMA
Host: negsum algebra in f64, ln, pos, threshold, mean -> scalar.
"""

import numpy as np

import concourse.bass as bass
import concourse.tile as tile
from concourse import mybir
from concourse.bass_utils import run_bass_kernel_spmd

# ---- problem constants (hardcoded per spec) ----
TEMP = 0.08
EPS = 1e-8
GAMMA = 0.99
BETA = 0.5 * (1.0 - GAMMA)
B, D, C = 8192, 256, 7
N = B + C                      # 8199 rows/cols of the score matrix
NCORES = 8
NT = 8                         # full row-tiles per core (8*8*128 = 8192)
ROWS_PER_CORE = NT * 128       # 1024
NA = 6144                      # block-A width (non-own columns)
NBMAX = 1536                   # hard cap on block-B width (psum tile)
SUPER = 2048                   # ftA piece width (= first A-block half)
NSLOT = 6                      # accum slots per row-tile: A1..A4 B1 B2
WA = 1536                      # A-supertile exp width
T8W = 1536                     # per-core share of row-tile 64's columns
OUTW = 56                      # NT*NSLOT main accums + 3 t64 accums + pad
M0 = 12.5                      # constant stand-in for the global max
A_SCALE = 0.5 / float(np.float32(TEMP))
BIAS = (0.5 + EPS) / float(np.float32(TEMP)) - M0
EXPBIAS = float(np.exp(np.float64(np.float32(BIAS))))  # one pad col's exp

F32 = mybir.dt.float32
BF16 = mybir.dt.bfloat16
FP8 = mybir.dt.float8e4
FP8NP = mybir.dt.np(mybir.dt.float8e4)
ACTF = mybir.ActivationFunctionType


def _split_multi_waits(nc):
    """This container's walrus accepts only ONE sync wait per instruction;
    split extra waits into standalone single-wait EventSemaphore insts."""
    n_new = 0
    for func in nc.m.functions:
        for blk in func.blocks:
            new_insts = []
            for inst in blk.instructions:
                si = getattr(inst, "sync_info", None)
                waits = list(si.on_wait) if si and si.on_wait else []
                if len(waits) > 1:
                    for i, w in enumerate(waits[:-1]):
                        n_new += 1
                        ev = mybir.InstEventSemaphore(
                            name=f"{inst.name}-wsplit{i}",
                            engine=inst.engine,
                            ins=[],
                            outs=[],
                            sync_info=mybir.SyncInfo(on_wait=[w], on_update=[]),
                            bass_nofuse=True,
                        )
                        new_insts.append(ev)
                    si.on_wait = [waits[-1]]
                new_insts.append(inst)
            blk.instructions = new_insts
    return n_new


def _pack_kt(mat):
    """[cols, D] f32 -> [128, 2, cols] fp8 DoubleRow operand layout."""
    cols = mat.shape[0]
    return np.ascontiguousarray(
        mat.T.reshape(2, 128, cols).transpose(1, 0, 2)
    ).astype(FP8NP)


def _host_prep(features, labels, prototypes, momentums):
    features = np.asarray(features, dtype=np.float32)
    labels = np.asarray(labels).astype(np.int64)
    prototypes = np.asarray(prototypes, dtype=np.float32)
    momentums = np.asarray(momentums, dtype=np.float32)

    # ---- prototype update: closed form of the sequential EMA scan ----
    counts_feat = np.bincount(labels, minlength=C)
    rank = np.zeros(B, dtype=np.int64)
    seen = np.zeros(C, dtype=np.int64)
    for i, l in enumerate(labels):
        rank[i] = seen[l]
        seen[l] += 1
    w = BETA * (GAMMA ** (counts_feat[labels] - 1 - rank).astype(np.float64))
    S = np.zeros((C, B))
    S[labels, np.arange(B)] = w
    m_final = S @ features.astype(np.float64)
    wsum = np.bincount(labels, weights=w, minlength=C)
    m_final -= wsum[:, None] * prototypes.astype(np.float64)
    m_final += (GAMMA ** counts_feat.astype(np.float64))[:, None] * momentums.astype(
        np.float64
    )
    target = prototypes.astype(np.float64) + m_final
    q, _ = np.linalg.qr(target.T.astype(np.float32))
    new_protos = q.T.astype(np.float32)

    # ---- normalized, label-sorted gram operands ----
    feats = np.concatenate([features, new_protos], 0)
    labs = np.concatenate([labels, np.arange(C, dtype=np.int64)])
    nrm = np.linalg.norm(feats.astype(np.float64), axis=-1)
    fhat = feats.astype(np.float64) / nrm[:, None]
    perm = np.argsort(labs, kind="stable")
    fs = fhat[perm]
    ls = labs[perm]
    counts_all = np.bincount(ls, minlength=C)          # includes protos
    bounds = np.concatenate([[0], np.cumsum(counts_all)])  # class col ranges

    fs32 = fs.astype(np.float32)
    Y = np.zeros((D, C), dtype=np.float64)
    for c in range(C):
        Y[:, c] = fs[bounds[c]:bounds[c + 1]].sum(0)

    # ---- shared row-tile 64 (last 7 rows), column-split across cores ----
    t8pad = np.zeros((128, D), dtype=np.float32)
    t8pad[:7] = fs32[B:N]
    t8rows = _pack_kt(t8pad)
    chunk_cols = []  # class-pure 512-col chunks (global col indices)
    chunk_cls = []
    for c in range(C):
        cols = np.arange(bounds[c], bounds[c + 1])
        for o in range(0, len(cols), 512):
            chunk_cols.append(cols[o:o + 512])
            chunk_cls.append(c)
    n_cpc = T8W // 512  # chunks per core
    while len(chunk_cols) < NCORES * n_cpc:
        chunk_cols.append(np.zeros(0, dtype=np.int64))
        chunk_cls.append(-1)
    t8meta = []  # (class, n_pad) per chunk for the host-side combine
    t8cols_per_core = []
    for core in range(NCORES):
        colmat = np.zeros((T8W, D), dtype=np.float32)
        for j in range(n_cpc):
            ci = core * n_cpc + j
            cols = chunk_cols[ci]
            colmat[j * 512:j * 512 + len(cols)] = fs32[cols]
            t8meta.append((chunk_cls[ci], 512 - len(cols)))
        t8cols_per_core.append(_pack_kt(colmat))

    nb = int(counts_all.max())          # B-block width, trimmed to the data
    assert nb <= NBMAX
    # A-block width: what the neediest core requires (two-class cores hold
    # all non-own cols in A; one-class cores may spill up to nb into B2)
    need_a = []
    for core in range(NCORES):
        own = sorted(set(ls[core * ROWS_PER_CORE:(core + 1) * ROWS_PER_CORE]
                         .tolist()))
        nonown = N - sum(int(counts_all[c]) for c in own)
        need_a.append(nonown if len(own) == 2 else nonown - nb)
    na = (max(need_a) + 15) // 16 * 16
    assert na <= NA
    wa = (na // 4 + 15) // 16 * 16     # A supertile width (first 3 supers)
    npad = na + 2 * nb
    per_core = []
    layouts = []
    for core in range(NCORES):
        base = core * ROWS_PER_CORE
        row_ls = ls[base:base + ROWS_PER_CORE]
        own = sorted(set(int(x) for x in row_ls))
        assert len(own) <= 2, f"core {core} spans {len(own)} classes"
        c1 = own[0]
        c2 = own[1] if len(own) == 2 else -1

        own_mask = np.isin(ls, own)
        nonown_cols = np.nonzero(~own_mask)[0]
        c1_cols = np.arange(bounds[c1], bounds[c1 + 1])
        if c2 >= 0:
            c2_cols = np.arange(bounds[c2], bounds[c2 + 1])
            a_cols = nonown_cols
        else:
            a_cols = nonown_cols[:na]
            c2_cols = nonown_cols[na:]  # non-own spill
        assert len(a_cols) <= na and len(c1_cols) <= nb and len(c2_cols) <= nb
        pads = (na - len(a_cols), nb - len(c1_cols), nb - len(c2_cols))

        ftpad = np.zeros((npad, D), dtype=np.float32)
        ftpad[0:len(a_cols)] = fs32[a_cols]
        ftpad[na:na + len(c1_cols)] = fs32[c1_cols]
        ftpad[na + nb:na + nb + len(c2_cols)] = fs32[c2_cols]
        ft = _pack_kt(ftpad)

        per_core.append(
            {
                "m0": np.ascontiguousarray(np.concatenate(
                    [ft[:, :, 0:wa],
                     _pack_kt(fs32[base:base + ROWS_PER_CORE])], axis=2)),
                "ftAB": np.ascontiguousarray(ft[:, :, wa:na]),
                "ftBB": np.ascontiguousarray(ft[:, :, na:]),
                "t8": np.ascontiguousarray(
                    np.concatenate([t8cols_per_core[core], t8rows], axis=2)),
            }
        )
        layouts.append((c1, c2, pads))

    host = {
        "ls": ls, "bounds": bounds, "counts_all": counts_all, "fs": fs,
        "Y": Y, "t8meta": t8meta, "layouts": layouts, "nb": nb, "na": na,
        "wa": wa,
    }
    return per_core, host


def _build_graph(na, nb, wa):
    npad = na + 2 * nb
    wa4 = na - 3 * wa
    nc = bass.Bass()
    m0_d = nc.declare_dram_parameter(
        "m0", [128, 2, wa + ROWS_PER_CORE], FP8, isOutput=False
    )
    ftAB_d = nc.declare_dram_parameter(
        "ftAB", [128, 2, na - wa], FP8, isOutput=False
    )
    ftBB_d = nc.declare_dram_parameter(
        "ftBB", [128, 2, npad - na], FP8, isOutput=False
    )
    t8_d = nc.declare_dram_parameter("t8", [128, 2, T8W + 128], FP8, isOutput=False)
    out_d = nc.declare_dram_parameter("out", [128, OUTW], F32, isOutput=True)

    with tile.TileContext(nc) as tc:
        with (
            tc.tile_pool(name="persist", bufs=1) as persist,
            tc.tile_pool(name="ps", bufs=2, space="PSUM") as psA,
            tc.tile_pool(name="scr", bufs=2) as scrp,
        ):
            # --- resident inputs (contiguous per-partition = fat DMAs) ---
            # order: t64 operands first (fills the DMA shadow), then the
            # pieces in first-use order for tile 0 = A1..A4 B1 B2
            t8_sb = persist.tile([128, 2, T8W + 128], FP8, tag="t8")
            nc.sync.dma_start(out=t8_sb[:], in_=t8_d[:])
            t8c_sb = t8_sb[:, :, 0:T8W]
            t8r_sb = t8_sb[:, :, T8W:T8W + 128]
            m0_sb = persist.tile([128, 2, wa + ROWS_PER_CORE], FP8, tag="m0")
            nc.sync.dma_start(out=m0_sb[:], in_=m0_d[:])
            ftA_sb = m0_sb[:, :, 0:wa]
            rows_sb = m0_sb[:, :, wa:wa + ROWS_PER_CORE]
            ftAB_sb = persist.tile([128, 2, na - wa], FP8, tag="ftAB")
            nc.sync.dma_start(out=ftAB_sb[:], in_=ftAB_d[:])
            ftBB_sb = persist.tile([128, 2, npad - na], FP8, tag="ftBB")
            nc.sync.dma_start(out=ftBB_sb[:], in_=ftBB_d[:])

            bias_exp = persist.tile([128, 1], F32, tag="bias_exp")
            nc.vector.memset(bias_exp[:], float(BIAS))
            out_t = persist.tile([128, OUTW], F32, tag="out")

            # warm the ACT exp table inside the DMA shadow
            dummy = persist.tile([128, 1], F32, tag="dummy")
            nc.scalar.activation(dummy[:], bias_exp[:], ACTF.Exp, bias=0.0)

            def rhs(lo, hi):
                if hi <= wa:
                    return ftA_sb[:, :, lo:hi]
                if hi <= na:
                    return ftAB_sb[:, :, lo - wa:hi - wa]
                return ftBB_sb[:, :, lo - na:hi - na]

            # --- shared row-tile 64 first: fills the input-DMA shadow ---
            ps8 = psA.tile([128, WA], F32, tag="ps")
            for j in range(T8W // 512):
                nc.tensor.matmul(
                    ps8[:, j * 512:(j + 1) * 512],
                    lhsT=t8r_sb[:],
                    rhs=t8c_sb[:, :, j * 512:(j + 1) * 512],
                    start=True,
                    stop=True,
                    perf_mode=mybir.MatmulPerfMode.DoubleRow,
                )
            scr8 = scrp.tile([128, WA], BF16, tag="scr")
            for j in range(T8W // 512):
                nc.scalar.activation(
                    scr8[:, j * 512:(j + 1) * 512], ps8[:, j * 512:(j + 1) * 512],
                    ACTF.Exp, bias=bias_exp[:], scale=float(A_SCALE),
                    accum_out=out_t[:, NT * NSLOT + j:NT * NSLOT + j + 1],
                )

            # --- main loop over row-tiles ---
            # per tile: 6 supertiles  B1(1280) B2(1280) A(1536)x4 — every
            # matmul group (<=3 chunks) hides under every exp (>=1280 wide)
            sup_w = (nb, nb, wa, wa, wa, wa4)
            sup_lo = (na, na + nb, 0, wa, 2 * wa, 3 * wa)
            slot_of = (4, 5, 0, 1, 2, 3)  # accum slot per super (host order)
            for t in range(NT):
                lhsT = rows_sb[:, :, t * 128:(t + 1) * 128]
                order = (2, 3, 4, 5, 0, 1) if t == 0 else range(NSLOT)
                for s in order:
                    ps = psA.tile([128, WA], F32, tag="ps")
                    lo, w = sup_lo[s], sup_w[s]
                    for c in range(0, w, 512):
                        cw = min(512, w - c)
                        nc.tensor.matmul(
                            ps[:, c:c + cw],
                            lhsT=lhsT,
                            rhs=rhs(lo + c, lo + c + cw),
                            start=True,
                            stop=True,
                            perf_mode=mybir.MatmulPerfMode.DoubleRow,
                        )
                    scr = scrp.tile([128, WA], BF16, tag="scr")
                    k = t * NSLOT + slot_of[s]
                    nc.scalar.activation(
                        scr[:, 0:w], ps[:, 0:w], ACTF.Exp,
                        bias=bias_exp[:], scale=float(A_SCALE),
                        accum_out=out_t[:, k:k + 1],
                    )

            nc.sync.dma_start(out=out_d[:], in_=out_t[:])
    return nc


def _combine(results, host):
    """Host-side unshard: all O(N) math in f64."""
    ls = host["ls"]
    fs, Y = host["fs"], host["Y"]
    counts_all = host["counts_all"]
    cnt = (counts_all[ls] - 1).astype(np.float64)
    selfsim = (fs.astype(np.float64) ** 2).sum(1)

    # pos_i = (A*(f_i . Y_c - selfsim_i) + BIAS*cnt_i) / (cnt_i + EPS)
    pos_sel = np.einsum("id,di->i", fs, Y[:, ls])
    pos = (A_SCALE * (pos_sel - selfsim) + BIAS * cnt) / (cnt + EPS)

    # negsum for main rows from per-core accumulators
    negsum = np.zeros(N, dtype=np.float64)
    for core in range(NCORES):
        o = np.asarray(results[core]["out"], dtype=np.float64)
        acc = o[:, :NT * NSLOT].reshape(128, NT, NSLOT)
        c1, c2, (pa, pb1, pb2) = host["layouts"][core]
        base = core * ROWS_PER_CORE
        row_ls = ls[base:base + ROWS_PER_CORE].reshape(NT, 128).T  # [128, NT]
        sel1 = (row_ls != c1).astype(np.float64)
        sel2 = (
            np.ones_like(sel1) if c2 < 0 else (row_ls != c2).astype(np.float64)
        )
        ns = (
            acc[:, :, 0] + acc[:, :, 1] + acc[:, :, 2] + acc[:, :, 3]
            + sel1 * acc[:, :, 4] + sel2 * acc[:, :, 5]
            - (pa + sel1 * pb1 + sel2 * pb2) * EXPBIAS
        )
        negsum[base:base + ROWS_PER_CORE] = ns.T.reshape(-1)

    # row-tile 64: rows 8192..8198 — class sums from per-core chunk sums
    n7 = N - B
    n_cpc = T8W // 512
    classsum = np.zeros((n7, C), dtype=np.float64)
    for core in range(NCORES):
        o = np.asarray(results[core]["out"], dtype=np.float64)
        for j in range(n_cpc):
            cls, n_pad = host["t8meta"][core * n_cpc + j]
            if cls < 0:
                continue
            classsum[:, cls] += o[:n7, NT * NSLOT + j] - n_pad * EXPBIAS
    stot = classsum.sum(1)
    rows_ls = ls[B:N]
    negsum[B:N] = stot - classsum[np.arange(n7), rows_ls]

    loss = -pos + np.log(negsum + EPS)
    m = loss > 0
    cnt_sum = m.sum()
    val = loss[m].sum() / max(cnt_sum, 1.0) if cnt_sum > 0 else 0.0
    return np.float32(val)


def _run(features, labels, prototypes, momentums, trace=False, trace_kwargs=None):
    per_core, host = _host_prep(features, labels, prototypes, momentums)
    nc = _build_graph(host["na"], host["nb"], host["wa"])
    _split_multi_waits(nc)
    in_maps = [per_core[i] for i in range(NCORES)]
    kw = {}
    if trace:
        kw = dict(trace=True, trace_cores=list(range(NCORES)))
        if trace_kwargs:
            kw["trace_kwargs"] = trace_kwargs
    res = run_bass_kernel_spmd(nc, in_maps, core_ids=list(range(NCORES)), **kw)
    return _combine(res.results, host), res


def kernel(features, labels, prototypes, momentums):
    val, _ = _run(features, labels, prototypes, momentums)
    return np.array(val, dtype=np.float32)
